# revision 4
# baseline (speedup 1.0000x reference)
"""Trainium2 Bass kernel for causal multi-head self-attention with RoPE.

Module: x[1,4096,1024] -> MHA(16 heads, d_k=64, causal, interleaved-pair RoPE)
        -> out[1,4096,1024], all fp32.

Sharding: head-parallel across 8 NeuronCores (2 heads/core). Each core
computes Q/K/V projections for its 2 heads, causal attention, and a partial
output projection against its column slice of Wo; the host sums the 8
partials (the tensor-parallel all-reduce equivalent).

Compute dtype: fp16 operands with fp32 PSUM accumulation (PE runs fp16 at
1 col/cycle vs fp32's 4; measured matmul rms rel err 2.9e-4).  The softmax
denominator chain (sums row -> reciprocal -> broadcast) stays fp32.

Device algorithm (per core):
  - Host passes x pre-transposed (xT [1024,4096] fp16) and weight slices
    pre-laid out for the PE's lhsT convention; 1/sqrt(d_k) folded into Wq.
  - Q^T,K^T [128,4096] (2 heads stacked on partitions) via 8 accumulating
    matmuls per 512-col tile; V in natural [t,d] layout via xT-stationary
    matmuls.  RoPE on DVE with host-built cos/sin tables; the pair mix is a
    32-partition block swap (W rows host-permuted so each head's dims are
    [even | odd]).
  - Scores computed transposed: S^T[k,q] = KT-slice-stationary matmul with
    QT moving, both heads concurrently via PE row groups (d_k=64).  exp on
    ScalarE (no max subtraction: scores are O(+-8), safe in fp32), causal
    handled by skipping k-tiles, column-slicing diagonal tiles, and one
    128x128 triangular mask multiply.
  - P@V with stationary [V | ones]: the ones column makes the softmax
    denominators ride along as psum row 64.  ctx^T accumulates in PSUM.
  - Normalization: reciprocal_approx on the sums row, broadcast across
    partitions via K=1 outer-product matmuls, fused into the PSUM->SBUF
    evacuation multiplies.
  - Output projection out^T[m,t] = WoT-stationary matmuls; fp16 partial
    out^T [1024,4096] DMA'd to DRAM; host sums partials in fp32.
"""

import sys

if "/opt/trn_rl_repo" not in sys.path:
    sys.path.insert(0, "/opt/trn_rl_repo")

import numpy as np

S = 4096
D = 1024
NHEADS = 16
DK = 64
NCORES = 8
HPC = NHEADS // NCORES  # 2 heads per core
TT = 512  # t/q tile width
NT = S // TT  # 8 tiles
KT = 128  # k tile width
THETA = 10000.0


# ----------------------------------------------------------------------------
# Host-side input preparation
# ----------------------------------------------------------------------------

def _perm64():
    """Within-head dim permutation: even dims first, then odd dims."""
    return np.concatenate([np.arange(0, DK, 2), np.arange(1, DK, 2)])


def _chunk_lhsT(w_rows):
    """[128 rows, 1024 e] weight slice -> [128 e_local, 8*128] lhsT layout.

    Result[:, 128c:128c+128] = w_rows[:, 128c:128c+128].T, i.e. the
    8 contraction-chunk stationary operands laid side by side.
    """
    t = np.ascontiguousarray(w_rows.T)  # [1024 e, 128 d]
    return np.ascontiguousarray(
        np.transpose(t.reshape(8, 128, 128), (1, 0, 2)).reshape(128, 1024)
    )


def _rope_tables():
    inv_freq = THETA ** (-np.arange(0, DK, 2, dtype=np.float64) / DK)  # [32]
    ang = np.arange(S, dtype=np.float64)[None, :] * inv_freq[:, None]  # [32, S]
    cos32 = np.cos(ang)
    sin32 = np.sin(ang)
    # rows: [h0x1(0:32) h0x2(32:64) h1x1(64:96) h1x2(96:128)]
    cos = np.concatenate([cos32, cos32, cos32, cos32], 0)
    # sin table is indexed at SOURCE rows of the swapped read:
    #  x1-block rows carry +sin (their product lands on the x2 output rows),
    #  x2-block rows carry -sin (landing on the x1 output rows).
    sinp = np.concatenate([sin32, -sin32, sin32, -sin32], 0)
    return cos, sinp


def prepare_core_inputs(x, Wq, Wk, Wv, Wo, core):
    """Build the per-core device input map (fp16 compute operands)."""
    x, Wq, Wk, Wv, Wo = (np.asarray(a) for a in (x, Wq, Wk, Wv, Wo))
    p64 = _perm64()
    h0, h1 = HPC * core, HPC * core + 1
    rows_perm = np.concatenate([h0 * DK + p64, h1 * DK + p64])  # 128 perm'd rows
    rows_nat = np.arange(HPC * DK * core, HPC * DK * (core + 1))  # 128 natural

    wq_eff = Wq[rows_perm, :].astype(np.float64) / np.sqrt(DK)
    wk_eff = Wk[rows_perm, :].astype(np.float64)
    wv_eff = Wv[rows_nat, :].astype(np.float64)
    cos, sinp = _rope_tables()

    tri = np.triu(np.ones((128, 128), dtype=np.float16))  # 1 where k<=q
    xT = np.ascontiguousarray(x[0].T).astype(np.float16)  # [1024, 4096]

    return {
        "xT": xT,
        "wq": _chunk_lhsT(wq_eff).astype(np.float16),
        "wk": _chunk_lhsT(wk_eff).astype(np.float16),
        "wv": _chunk_lhsT(wv_eff).astype(np.float16),
        "wo": np.ascontiguousarray(Wo[:, rows_nat].T).astype(np.float16),
        "cosT": cos.astype(np.float16),
        "sinT": sinp.astype(np.float16),
        "tri": tri,
        "one64": np.ones((1, 64), dtype=np.float16),
    }


# ----------------------------------------------------------------------------
# Numpy emulation of the device dataflow (fast layout + precision validation)
# ----------------------------------------------------------------------------

def emulate_core(ins):
    """Mirror the device algorithm (incl. fp16 roundings) in numpy."""
    h16 = lambda a: a.astype(np.float16).astype(np.float32)
    xT = ins["xT"].astype(np.float32)
    cos, sinp = (ins["cosT"].astype(np.float32), ins["sinT"].astype(np.float32))

    def proj_T(w_lhsT):
        w = w_lhsT.astype(np.float32)
        out = np.zeros((128, S), np.float32)
        for c in range(8):
            out += w[:, 128 * c:128 * (c + 1)].T @ xT[128 * c:128 * (c + 1), :]
        return out

    def rope(ps):
        raw = h16(ps)
        t1 = h16(raw * cos)
        u = h16(raw * sinp)
        t2 = np.empty_like(raw)
        for b in range(4):
            s = b ^ 1
            t2[32 * b:32 * b + 32] = u[32 * s:32 * s + 32]
        return h16(t1 + t2)

    qt = rope(proj_T(ins["wq"]))  # [128, S] fp16-valued
    kt = rope(proj_T(ins["wk"]))
    wv = ins["wv"].astype(np.float32)
    v = np.zeros((S, 128), np.float32)
    for c in range(8):
        v += xT[128 * c:128 * (c + 1), :].T @ wv[:, 128 * c:128 * (c + 1)]
    v = h16(v)
    tri = ins["tri"].astype(np.float32)

    outT = np.zeros((D, S), np.float32)
    wo = ins["wo"].astype(np.float32)
    for j in range(NT):
        qs = slice(TT * j, TT * (j + 1))
        ctxn = np.zeros((128, TT), np.float32)
        for h in range(2):
            hq = qt[64 * h:64 * h + 64, qs]
            ctx = np.zeros((65, TT), np.float32)
            for t in range(4 * (j + 1)):
                ks = slice(KT * t, KT * (t + 1))
                dlt = t - 4 * j
                st = kt[64 * h:64 * h + 64, ks].T @ hq  # [128 k, 512 q] fp32
                pt = h16(np.exp(st))
                off = 0
                if dlt >= 0:
                    off = 128 * dlt
                    pt[:, off:off + 128] = h16(pt[:, off:off + 128] * tri)
                vplus = np.concatenate(
                    [v[KT * t:KT * (t + 1), 64 * h:64 * h + 64],
                     np.ones((128, 1), np.float32)], 1)
                ctx[:, off:] += vplus.T @ pt[:, off:]
            recip = h16(1.0 / ctx[64])
            ctxn[64 * h:64 * h + 64] = h16(ctx[0:64] * recip[None, :])
        outT[:, qs] = h16(wo.T @ ctxn)
    return outT


def emulate(x, Wq, Wk, Wv, Wo):
    acc = np.zeros((D, S), dtype=np.float64)
    for core in range(NCORES):
        acc += emulate_core(prepare_core_inputs(x, Wq, Wk, Wv, Wo, core))
    return np.ascontiguousarray(acc.T.astype(np.float32))[None, :, :]


# ----------------------------------------------------------------------------
# Bass kernel
# ----------------------------------------------------------------------------

def build_nc():
    import concourse.bacc as bacc
    import concourse.mybir as mybir
    import concourse.tile as tile

    f32 = mybir.dt.float32
    f16 = mybir.dt.float16
    AF = mybir.ActivationFunctionType

    nc = bacc.Bacc("TRN2", target_bir_lowering=False, debug=False,
                   num_devices=NCORES)

    xT_d = nc.dram_tensor("xT", [D, S], f16, kind="ExternalInput")
    wq_d = nc.dram_tensor("wq", [128, 1024], f16, kind="ExternalInput")
    wk_d = nc.dram_tensor("wk", [128, 1024], f16, kind="ExternalInput")
    wv_d = nc.dram_tensor("wv", [128, 1024], f16, kind="ExternalInput")
    wo_d = nc.dram_tensor("wo", [128, 1024], f16, kind="ExternalInput")
    cos_d = nc.dram_tensor("cosT", [128, S], f16, kind="ExternalInput")
    sin_d = nc.dram_tensor("sinT", [128, S], f16, kind="ExternalInput")
    tri_d = nc.dram_tensor("tri", [128, 128], f16, kind="ExternalInput")
    one_d = nc.dram_tensor("one64", [1, 64], f16, kind="ExternalInput")
    out_d = nc.dram_tensor("outT", [D, S], f16, kind="ExternalOutput")

    with tile.TileContext(nc) as tc:
        with (
            tc.tile_pool(name="const", bufs=1) as const,
            tc.tile_pool(name="xt", bufs=2) as xtp,
            tc.tile_pool(name="work", bufs=2) as work,
            tc.tile_pool(name="pt", bufs=3) as ptp,
            tc.tile_pool(name="stage", bufs=3) as stage,
            tc.tile_pool(name="small", bufs=2) as small,
            tc.tile_pool(name="ps_s", bufs=2, space="PSUM") as ps_s,
            tc.tile_pool(name="ps_ctx", bufs=1, space="PSUM") as ps_ctx,
            tc.tile_pool(name="ps_misc", bufs=2, space="PSUM") as ps_misc,
        ):
            # ---- constants ----
            wq_sb = const.tile([128, 1024], f16, tag="wq")
            wk_sb = const.tile([128, 1024], f16, tag="wk")
            wv_sb = const.tile([128, 1024], f16, tag="wv")
            wo_sb = const.tile([128, 1024], f16, tag="wo")
            cos_sb = const.tile([128, S], f16, tag="cos")
            sin_sb = const.tile([128, S], f16, tag="sin")
            tri_sb = const.tile([128, 128], f16, tag="tri")
            one_sb = const.tile([1, 64], f16, tag="one")
            for sb, dr in ((wq_sb, wq_d), (wk_sb, wk_d), (wv_sb, wv_d),
                           (wo_sb, wo_d), (cos_sb, cos_d), (sin_sb, sin_d),
                           (tri_sb, tri_d), (one_sb, one_d)):
                nc.sync.dma_start(sb[:], dr[:])

            # persistent activations
            qt_sb = const.tile([128, S], f16, tag="qt")
            kt_sb = const.tile([128, S], f16, tag="kt")
            # V + ones column, per k-tile slot: [128, slot, head, 65]
            v_sb = const.tile([128, S // KT, 2, 65], f16, tag="v")
            nc.vector.memset(v_sb[:, :, :, 64:65], 1.0)

            # ---- phase 1: projections ----
            def rope(raw, dst, ts):
                t1 = work.tile([128, TT], f16, tag="t1")
                nc.vector.tensor_mul(t1[:], raw[:], cos_sb[:, ts])
                t2 = work.tile([128, TT], f16, tag="t2")
                for b in range(4):
                    sblk = b ^ 1
                    nc.vector.tensor_mul(
                        t2[32 * b:32 * b + 32],
                        raw[32 * sblk:32 * sblk + 32],
                        sin_sb[32 * sblk:32 * sblk + 32, ts],
                    )
                nc.vector.tensor_add(dst, t1[:], t2[:])

            for j in range(NT):
                ts = slice(TT * j, TT * (j + 1))
                xt = xtp.tile([128, 8, TT], f16, tag="xt")
                for c in range(8):
                    nc.sync.dma_start(xt[:, c, :], xT_d[128 * c:128 * (c + 1), ts])

                for w_sb, dst in ((wq_sb, qt_sb), (wk_sb, kt_sb)):
                    ps = ps_misc.tile([128, TT], f32, tag="m")
                    for c in range(8):
                        nc.tensor.matmul(ps[:], w_sb[:, 128 * c:128 * (c + 1)],
                                         xt[:, c, :], start=(c == 0), stop=(c == 7))
                    raw = work.tile([128, TT], f16, tag="raw")
                    nc.scalar.copy(raw[:], ps[:])
                    rope(raw, dst[:, ts], ts)

                for s4 in range(4):
                    psv = ps_misc.tile([128, 128], f32, tag="m")
                    for c in range(8):
                        nc.tensor.matmul(
                            psv[:],
                            xt[:, c, 128 * s4:128 * (s4 + 1)],
                            wv_sb[:, 128 * c:128 * (c + 1)],
                            start=(c == 0), stop=(c == 7),
                        )
                    slot = 4 * j + s4
                    nc.vector.tensor_copy(
                        v_sb[:, slot, :, 0:64],
                        psv.rearrange("p (h d) -> p h d", h=2),
                    )

            # ---- phase 2: attention + finalize per q-tile ----
            AFexp = AF.Exp
            for j in range(NT):
                qs = slice(TT * j, TT * (j + 1))
                n_k = 4 * (j + 1)
                ctx0 = ps_ctx.tile([65, TT], f32, tag="ctx0")
                ctx1 = ps_ctx.tile([65, TT], f32, tag="ctx1")
                for t in range(n_k):
                    ks = slice(KT * t, KT * (t + 1))
                    dlt = t - 4 * j
                    off = 128 * dlt if dlt >= 0 else 0
                    pss = ps_s.tile([128, 2 * TT], f32, tag="s")
                    nc.tensor.matmul(pss[:, 0:TT], kt_sb[0:64, ks],
                                     qt_sb[0:64, qs], start=True, stop=True)
                    nc.tensor.matmul(pss[:, TT:2 * TT], kt_sb[64:128, ks],
                                     qt_sb[64:128, qs], start=True, stop=True)
                    pt = ptp.tile([128, 2 * TT], f16, tag="pt")
                    nc.scalar.activation(pt[:], pss[:], AFexp)
                    if dlt >= 0:
                        for h in range(2):
                            seg = slice(TT * h + off, TT * h + off + 128)
                            nc.vector.tensor_mul(pt[:, seg], pt[:, seg], tri_sb[:])
                    for h, ctx in ((0, ctx0), (1, ctx1)):
                        nc.tensor.matmul(
                            ctx[:, off:TT],
                            v_sb[:, t, h, :],
                            pt[:, TT * h + off:TT * (h + 1)],
                            start=(t == 0), stop=(t == n_k - 1),
                        )

                # normalize + output projection for this q-tile
                sm0 = small.tile([1, TT], f32, tag="sm0")
                sm1 = small.tile([1, TT], f32, tag="sm1")
                nc.vector.tensor_copy(sm0[:], ctx0[64:65, :])
                nc.vector.tensor_copy(sm1[:], ctx1[64:65, :])
                scr = small.tile([1, TT], f32, tag="scr")
                rec0 = small.tile([1, TT], f32, tag="rec0")
                rec1 = small.tile([1, TT], f32, tag="rec1")
                nc.vector.reciprocal_approx_accurate(rec0[:], sm0[:], scr[:])
                nc.vector.reciprocal_approx_accurate(rec1[:], sm1[:], scr[:])
                rh0 = small.tile([1, TT], f16, tag="rh0")
                rh1 = small.tile([1, TT], f16, tag="rh1")
                nc.vector.tensor_copy(rh0[:], rec0[:])
                nc.vector.tensor_copy(rh1[:], rec1[:])
                bc0 = ps_misc.tile([64, TT], f32, tag="m")
                bc1 = ps_misc.tile([64, TT], f32, tag="m")
                nc.tensor.matmul(bc0[:], one_sb[:], rh0[:], start=True, stop=True)
                nc.tensor.matmul(bc1[:], one_sb[:], rh1[:], start=True, stop=True)
                bcs0 = stage.tile([64, TT], f32, tag="bcs0")
                bcs1 = stage.tile([64, TT], f32, tag="bcs1")
                nc.scalar.copy(bcs0[:], bc0[:])
                nc.scalar.copy(bcs1[:], bc1[:])
                ctxn = stage.tile([128, TT], f16, tag="ctxn")
                nc.vector.tensor_mul(ctxn[0:64, :], ctx0[0:64, :], bcs0[:])
                nc.vector.tensor_mul(ctxn[64:128, :], ctx1[0:64, :], bcs1[:])

                for m in range(8):
                    po = ps_misc.tile([128, TT], f32, tag="m")
                    nc.tensor.matmul(po[:], wo_sb[:, 128 * m:128 * (m + 1)],
                                     ctxn[:], start=True, stop=True)
                    ot = stage.tile([128, TT], f16, tag="ot")
                    nc.any.tensor_copy(ot[:], po[:])
                    nc.sync.dma_start(out_d[128 * m:128 * (m + 1), qs], ot[:])

    nc.compile()
    return nc


_NC_CACHE = {}


def kernel(x, Wq, Wk, Wv, Wo):
    from concourse.bass_utils import run_bass_kernel_spmd

    if "nc" not in _NC_CACHE:
        _NC_CACHE["nc"] = build_nc()
    nc = _NC_CACHE["nc"]

    in_maps = [prepare_core_inputs(x, Wq, Wk, Wv, Wo, c) for c in range(NCORES)]
    res = run_bass_kernel_spmd(nc, in_maps, core_ids=list(range(NCORES)))
    acc = np.zeros((D, S), dtype=np.float32)
    for r in res.results:
        acc += r["outT"].astype(np.float32)
    return np.ascontiguousarray(acc.T)[None, :, :].astype(np.float32)


# revision 10
# speedup vs baseline: 24.2381x; 24.2381x over previous
"""Trainium2 Bass kernel for causal multi-head self-attention with RoPE.

Module: x[1,4096,1024] -> MHA(16 heads, d_k=64, causal, interleaved-pair RoPE)
        -> out[1,4096,1024], all fp32.

Sharding: head-parallel across 8 NeuronCores (2 heads/core). Each core
computes Q/K/V projections for its 2 heads, causal attention, and a partial
output projection against its column slice of Wo; the host sums the 8
partials (the tensor-parallel all-reduce equivalent).

Compute dtype: fp16 operands with fp32 PSUM accumulation (PE runs fp16 at
1 col/cycle vs fp32's 4; measured matmul rms rel err 2.9e-4).  The softmax
denominator chain (sums row -> reciprocal -> broadcast) stays fp32.

Device algorithm (per core):
  - Host passes x pre-transposed (xT [1024,4096] fp16) and weight slices
    pre-laid out for the PE's lhsT convention; 1/sqrt(d_k) folded into Wq.
  - Q^T,K^T [128,4096] (2 heads stacked on partitions) via 8 accumulating
    matmuls per 512-col tile; V in natural [t,d] layout via xT-stationary
    matmuls.  RoPE on DVE with host-built cos/sin tables; the pair mix is a
    32-partition block swap (W rows host-permuted so each head's dims are
    [even | odd]).
  - Scores computed transposed: S^T[k,q] = KT-slice-stationary matmul with
    QT moving, both heads concurrently via PE row groups (d_k=64).  exp on
    ScalarE (no max subtraction: scores are O(+-8), safe in fp32), causal
    handled by skipping k-tiles, column-slicing diagonal tiles, and one
    128x128 triangular mask multiply.
  - P@V with stationary [V | ones]: the ones column makes the softmax
    denominators ride along as psum row 64.  ctx^T accumulates in PSUM.
  - Normalization: reciprocal_approx on the sums row, broadcast across
    partitions via K=1 outer-product matmuls, fused into the PSUM->SBUF
    evacuation multiplies.
  - Output projection out^T[m,t] = WoT-stationary matmuls; fp16 partial
    out^T [1024,4096] DMA'd to DRAM; host sums partials in fp32.
"""

import sys

if "/opt/trn_rl_repo" not in sys.path:
    sys.path.insert(0, "/opt/trn_rl_repo")

import numpy as np

S = 4096
D = 1024
NHEADS = 16
DK = 64
NCORES = 8
HPC = NHEADS // NCORES  # 2 heads per core
TT = 512  # t/q tile width
NT = S // TT  # 8 tiles
KT = 128  # k tile width
THETA = 10000.0


# ----------------------------------------------------------------------------
# Host-side input preparation
# ----------------------------------------------------------------------------

def _perm64():
    """Within-head dim permutation: even dims first, then odd dims."""
    return np.concatenate([np.arange(0, DK, 2), np.arange(1, DK, 2)])


def _chunk_lhsT(w_rows):
    """[128 rows, 1024 e] weight slice -> [128 e_local, 8*128] lhsT layout.

    Result[:, 128c:128c+128] = w_rows[:, 128c:128c+128].T, i.e. the
    8 contraction-chunk stationary operands laid side by side.
    """
    t = np.ascontiguousarray(w_rows.T)  # [1024 e, 128 d]
    return np.ascontiguousarray(
        np.transpose(t.reshape(8, 128, 128), (1, 0, 2)).reshape(128, 1024)
    )


def _rope_tables():
    inv_freq = THETA ** (-np.arange(0, DK, 2, dtype=np.float64) / DK)  # [32]
    ang = np.arange(S, dtype=np.float64)[None, :] * inv_freq[:, None]  # [32, S]
    cos32 = np.cos(ang)
    sin32 = np.sin(ang)
    # rows: [h0x1(0:32) h0x2(32:64) h1x1(64:96) h1x2(96:128)]
    cos = np.concatenate([cos32, cos32, cos32, cos32], 0)
    # sin table is indexed at SOURCE rows of the swapped read:
    #  x1-block rows carry +sin (their product lands on the x2 output rows),
    #  x2-block rows carry -sin (landing on the x1 output rows).
    sinp = np.concatenate([sin32, -sin32, sin32, -sin32], 0)
    return cos, sinp


def prepare_core_inputs(x, Wq, Wk, Wv, Wo, core):
    """Build the per-core device input map (fp16 compute operands)."""
    x, Wq, Wk, Wv, Wo = (np.asarray(a) for a in (x, Wq, Wk, Wv, Wo))
    p64 = _perm64()
    h0, h1 = HPC * core, HPC * core + 1
    rows_perm = np.concatenate([h0 * DK + p64, h1 * DK + p64])  # 128 perm'd rows
    rows_nat = np.arange(HPC * DK * core, HPC * DK * (core + 1))  # 128 natural

    wq_eff = Wq[rows_perm, :].astype(np.float64) / np.sqrt(DK)
    wk_eff = Wk[rows_perm, :].astype(np.float64)
    wv_eff = Wv[rows_nat, :].astype(np.float64)
    cos, sinp = _rope_tables()

    tri = np.triu(np.ones((128, 128), dtype=np.float16))  # 1 where k<=q
    xT = np.ascontiguousarray(x[0].T).astype(np.float16)  # [1024, 4096]

    return {
        "xT": xT,
        "wq": _chunk_lhsT(wq_eff).astype(np.float16),
        "wk": _chunk_lhsT(wk_eff).astype(np.float16),
        "wv": _chunk_lhsT(wv_eff).astype(np.float16),
        "wo": np.ascontiguousarray(Wo[:, rows_nat].T).astype(np.float16),
        "cosT": cos.astype(np.float16),
        "sinT": sinp.astype(np.float16),
        "tri": tri,
        "one64": np.ones((1, 64), dtype=np.float16),
    }


# ----------------------------------------------------------------------------
# Numpy emulation of the device dataflow (fast layout + precision validation)
# ----------------------------------------------------------------------------

def emulate_core(ins):
    """Mirror the device algorithm (incl. fp16 roundings) in numpy."""
    h16 = lambda a: a.astype(np.float16).astype(np.float32)
    xT = ins["xT"].astype(np.float32)
    cos, sinp = (ins["cosT"].astype(np.float32), ins["sinT"].astype(np.float32))

    def proj_T(w_lhsT):
        w = w_lhsT.astype(np.float32)
        out = np.zeros((128, S), np.float32)
        for c in range(8):
            out += w[:, 128 * c:128 * (c + 1)].T @ xT[128 * c:128 * (c + 1), :]
        return out

    def rope(ps):
        raw = h16(ps)
        t1 = h16(raw * cos)
        u = h16(raw * sinp)
        t2 = np.empty_like(raw)
        for b in range(4):
            s = b ^ 1
            t2[32 * b:32 * b + 32] = u[32 * s:32 * s + 32]
        return h16(t1 + t2)

    qt = rope(proj_T(ins["wq"]))  # [128, S] fp16-valued
    kt = rope(proj_T(ins["wk"]))
    wv = ins["wv"].astype(np.float32)
    v = np.zeros((S, 128), np.float32)
    for c in range(8):
        v += xT[128 * c:128 * (c + 1), :].T @ wv[:, 128 * c:128 * (c + 1)]
    v = h16(v)
    tri = ins["tri"].astype(np.float32)

    outT = np.zeros((D, S), np.float32)
    wo = ins["wo"].astype(np.float32)
    for j in range(NT):
        qs = slice(TT * j, TT * (j + 1))
        ctxn = np.zeros((128, TT), np.float32)
        for h in range(2):
            hq = qt[64 * h:64 * h + 64, qs]
            ctx = np.zeros((65, TT), np.float32)
            for t in range(4 * (j + 1)):
                ks = slice(KT * t, KT * (t + 1))
                dlt = t - 4 * j
                st = kt[64 * h:64 * h + 64, ks].T @ hq  # [128 k, 512 q] fp32
                pt = h16(np.exp(st))
                off = 0
                if dlt >= 0:
                    off = 128 * dlt
                    pt[:, off:off + 128] = h16(pt[:, off:off + 128] * tri)
                vplus = np.concatenate(
                    [v[KT * t:KT * (t + 1), 64 * h:64 * h + 64],
                     np.ones((128, 1), np.float32)], 1)
                ctx[:, off:] += vplus.T @ pt[:, off:]
            recip = h16(1.0 / ctx[64])
            ctxn[64 * h:64 * h + 64] = h16(ctx[0:64] * recip[None, :])
        outT[:, qs] = h16(wo.T @ ctxn)
    return outT


def emulate(x, Wq, Wk, Wv, Wo):
    acc = np.zeros((D, S), dtype=np.float64)
    for core in range(NCORES):
        acc += emulate_core(prepare_core_inputs(x, Wq, Wk, Wv, Wo, core))
    return np.ascontiguousarray(acc.T.astype(np.float32))[None, :, :]


# ----------------------------------------------------------------------------
# Bass kernel
# ----------------------------------------------------------------------------

def build_nc(loop_n=1):
    """Build the kernel; loop_n>1 wraps the body in a hardware For_i loop
    (identical per-iteration work) purely for slope-based device timing."""
    import contextlib

    import concourse.bacc as bacc
    import concourse.mybir as mybir
    import concourse.tile as tile

    f32 = mybir.dt.float32
    f16 = mybir.dt.float16
    AF = mybir.ActivationFunctionType

    nc = bacc.Bacc("TRN2", target_bir_lowering=False, debug=False,
                   num_devices=NCORES)

    xT_d = nc.dram_tensor("xT", [D, S], f16, kind="ExternalInput")
    wq_d = nc.dram_tensor("wq", [128, 1024], f16, kind="ExternalInput")
    wk_d = nc.dram_tensor("wk", [128, 1024], f16, kind="ExternalInput")
    wv_d = nc.dram_tensor("wv", [128, 1024], f16, kind="ExternalInput")
    wo_d = nc.dram_tensor("wo", [128, 1024], f16, kind="ExternalInput")
    cos_d = nc.dram_tensor("cosT", [128, S], f16, kind="ExternalInput")
    sin_d = nc.dram_tensor("sinT", [128, S], f16, kind="ExternalInput")
    tri_d = nc.dram_tensor("tri", [128, 128], f16, kind="ExternalInput")
    one_d = nc.dram_tensor("one64", [1, 64], f16, kind="ExternalInput")
    out_d = nc.dram_tensor("outT", [D, S], f16, kind="ExternalOutput")

    with tile.TileContext(nc) as tc:
        with (
            tc.tile_pool(name="const", bufs=1) as const,
            tc.tile_pool(name="xt", bufs=2) as xtp,
            tc.tile_pool(name="work", bufs=2) as work,
            tc.tile_pool(name="pt", bufs=4) as ptp,
            tc.tile_pool(name="stage", bufs=3) as stage,
            tc.tile_pool(name="small", bufs=2) as small,
            tc.tile_pool(name="ps_s", bufs=2, space="PSUM") as ps_s,
            tc.tile_pool(name="ps_ctx", bufs=1, space="PSUM") as ps_ctx,
            tc.tile_pool(name="ps_misc", bufs=2, space="PSUM") as ps_misc,
            (tc.For_i(0, loop_n, 1) if loop_n > 1
             else contextlib.nullcontext()),
        ):
            # ---- constants ----
            wq_sb = const.tile([128, 1024], f16, tag="wq")
            wk_sb = const.tile([128, 1024], f16, tag="wk")
            wv_sb = const.tile([128, 1024], f16, tag="wv")
            wo_sb = const.tile([128, 1024], f16, tag="wo")
            cos_sb = const.tile([128, S], f16, tag="cos")
            sin_sb = const.tile([128, S], f16, tag="sin")
            tri_sb = const.tile([128, 128], f16, tag="tri")
            one_sb = const.tile([1, 64], f16, tag="one")
            for sb, dr in ((wq_sb, wq_d), (wk_sb, wk_d), (wv_sb, wv_d),
                           (wo_sb, wo_d), (cos_sb, cos_d), (sin_sb, sin_d),
                           (tri_sb, tri_d), (one_sb, one_d)):
                nc.sync.dma_start(sb[:], dr[:])

            # persistent activations
            qt_sb = const.tile([128, S], f16, tag="qt")
            kt_sb = const.tile([128, S], f16, tag="kt")
            # V + ones column, per k-tile slot: [128, slot, head, 65]
            v_sb = const.tile([128, S // KT, 2, 65], f16, tag="v")
            nc.vector.memset(v_sb[:, :, :, 64:65], 1.0)

            # ---- phase 1: projections ----
            def rope(raw, dst, ts):
                t1 = work.tile([128, TT], f16, tag="t1")
                nc.vector.tensor_mul(t1[:], raw[:], cos_sb[:, ts])
                t2 = work.tile([128, TT], f16, tag="t2")
                for b in range(4):
                    sblk = b ^ 1
                    nc.vector.tensor_mul(
                        t2[32 * b:32 * b + 32],
                        raw[32 * sblk:32 * sblk + 32],
                        sin_sb[32 * sblk:32 * sblk + 32, ts],
                    )
                nc.vector.tensor_add(dst, t1[:], t2[:])

            for j in range(NT):
                ts = slice(TT * j, TT * (j + 1))
                xt = xtp.tile([128, 8, TT], f16, tag="xt")
                for c in range(8):
                    nc.sync.dma_start(xt[:, c, :], xT_d[128 * c:128 * (c + 1), ts])

                for w_sb, dst in ((wq_sb, qt_sb), (wk_sb, kt_sb)):
                    ps = ps_misc.tile([128, TT], f32, tag="m")
                    for c in range(8):
                        nc.tensor.matmul(ps[:], w_sb[:, 128 * c:128 * (c + 1)],
                                         xt[:, c, :], start=(c == 0), stop=(c == 7))
                    raw = work.tile([128, TT], f16, tag="raw")
                    nc.scalar.copy(raw[:], ps[:])
                    rope(raw, dst[:, ts], ts)

                for s4 in range(4):
                    psv = ps_misc.tile([128, 128], f32, tag="m")
                    for c in range(8):
                        nc.tensor.matmul(
                            psv[:],
                            xt[:, c, 128 * s4:128 * (s4 + 1)],
                            wv_sb[:, 128 * c:128 * (c + 1)],
                            start=(c == 0), stop=(c == 7),
                        )
                    slot = 4 * j + s4
                    nc.vector.tensor_copy(
                        v_sb[:, slot, :, 0:64],
                        psv.rearrange("p (h d) -> p h d", h=2),
                    )

            # ---- phase 2: attention + finalize per q-tile ----
            AFexp = AF.Exp
            for j in range(NT):
                qs = slice(TT * j, TT * (j + 1))
                n_k = 4 * (j + 1)
                ctx0 = ps_ctx.tile([65, TT], f32, tag="ctx0")
                ctx1 = ps_ctx.tile([65, TT], f32, tag="ctx1")
                for t in range(n_k):
                    ks = slice(KT * t, KT * (t + 1))
                    dlt = t - 4 * j
                    off = 128 * dlt if dlt >= 0 else 0
                    pss = ps_s.tile([128, 2 * TT], f32, tag="s")
                    qsl = slice(TT * j + off, TT * (j + 1))
                    nc.tensor.matmul(pss[:, off:TT], kt_sb[0:64, ks],
                                     qt_sb[0:64, qsl], start=True, stop=True)
                    nc.tensor.matmul(pss[:, TT + off:2 * TT], kt_sb[64:128, ks],
                                     qt_sb[64:128, qsl], start=True, stop=True)
                    pt = ptp.tile([128, 2 * TT], f16, tag="pt")
                    if off:
                        nc.scalar.activation(pt[:, off:TT], pss[:, off:TT], AFexp)
                        nc.scalar.activation(pt[:, TT + off:2 * TT],
                                             pss[:, TT + off:2 * TT], AFexp)
                    else:
                        nc.scalar.activation(pt[:], pss[:], AFexp)
                    if dlt >= 0:
                        for h in range(2):
                            seg = slice(TT * h + off, TT * h + off + 128)
                            nc.vector.tensor_mul(pt[:, seg], pt[:, seg], tri_sb[:])
                    for h, ctx in ((0, ctx0), (1, ctx1)):
                        nc.tensor.matmul(
                            ctx[:, off:TT],
                            v_sb[:, t, h, :],
                            pt[:, TT * h + off:TT * (h + 1)],
                            start=(t == 0), stop=(t == n_k - 1),
                        )

                # normalize + output projection for this q-tile
                sm0 = small.tile([1, TT], f32, tag="sm0")
                sm1 = small.tile([1, TT], f32, tag="sm1")
                nc.vector.tensor_copy(sm0[:], ctx0[64:65, :])
                nc.vector.tensor_copy(sm1[:], ctx1[64:65, :])
                scr = small.tile([1, TT], f32, tag="scr")
                rec0 = small.tile([1, TT], f32, tag="rec0")
                rec1 = small.tile([1, TT], f32, tag="rec1")
                nc.vector.reciprocal_approx_accurate(rec0[:], sm0[:], scr[:])
                nc.vector.reciprocal_approx_accurate(rec1[:], sm1[:], scr[:])
                rh0 = small.tile([1, TT], f16, tag="rh0")
                rh1 = small.tile([1, TT], f16, tag="rh1")
                nc.vector.tensor_copy(rh0[:], rec0[:])
                nc.vector.tensor_copy(rh1[:], rec1[:])
                bc0 = ps_misc.tile([64, TT], f32, tag="m")
                bc1 = ps_misc.tile([64, TT], f32, tag="m")
                nc.tensor.matmul(bc0[:], one_sb[:], rh0[:], start=True, stop=True)
                nc.tensor.matmul(bc1[:], one_sb[:], rh1[:], start=True, stop=True)
                bcs0 = stage.tile([64, TT], f32, tag="bcs0")
                bcs1 = stage.tile([64, TT], f32, tag="bcs1")
                nc.scalar.copy(bcs0[:], bc0[:])
                nc.scalar.copy(bcs1[:], bc1[:])
                ctxn = stage.tile([128, TT], f16, tag="ctxn")
                nc.vector.tensor_mul(ctxn[0:64, :], ctx0[0:64, :], bcs0[:])
                nc.vector.tensor_mul(ctxn[64:128, :], ctx1[0:64, :], bcs1[:])

                for m in range(8):
                    po = ps_misc.tile([128, TT], f32, tag="m")
                    nc.tensor.matmul(po[:], wo_sb[:, 128 * m:128 * (m + 1)],
                                     ctxn[:], start=True, stop=True)
                    ot = stage.tile([128, TT], f16, tag="ot")
                    nc.any.tensor_copy(ot[:], po[:])
                    nc.sync.dma_start(out_d[128 * m:128 * (m + 1), qs], ot[:])

    nc.compile()
    return nc


_NC_CACHE = {}


def kernel(x, Wq, Wk, Wv, Wo):
    from concourse.bass_utils import run_bass_kernel_spmd

    if "nc" not in _NC_CACHE:
        _NC_CACHE["nc"] = build_nc()
    nc = _NC_CACHE["nc"]

    in_maps = [prepare_core_inputs(x, Wq, Wk, Wv, Wo, c) for c in range(NCORES)]
    res = run_bass_kernel_spmd(nc, in_maps, core_ids=list(range(NCORES)))
    acc = np.zeros((D, S), dtype=np.float32)
    for r in res.results:
        acc += r["outT"].astype(np.float32)
    return np.ascontiguousarray(acc.T)[None, :, :].astype(np.float32)
